# revision 11
# baseline (speedup 1.0000x reference)
"""Trainium2 Bass kernel for nn_BatchGraphEncoder (gnn_message_passing).

Math note: the reference's segment softmax uses B unique segment ids
(groups of size 1), so alpha == 1.0 bit-exactly.  The output reduces to
pure batch sums:

    out[:,   0:128] = sum_b h[b,:]      (broadcast over the N=512 rows)
    out[:, 128:256] = sum_b r[b,:]      (broadcast)
    out[:, 256:384] = sum_b t[b,:,:]    ([512, 128])

Memory-bound reduction over B=2048 dominated by reading t (512 MB).
B is sharded equally across the 8 cores (256 rows each); the 8 tiny
partials are summed on the host.

Per-core strategy (v2, TensorEngine reduction):
  * Rows go on SBUF partitions: stripe tiles [128 rows, 16384 cols] give
    64 KB contiguous DMA runs per partition (vs 2 KB for the
    columns-on-partitions layout), lifting the 16 SDMA engines from
    ~80 ns/2KB packet (~400 GB/s) to near the 435 GB/s fabric ceiling.
  * The batch sum contracts the partition dim, which is exactly what the
    PE does.  Each 512-column chunk c of the flattened [512*128] feature
    axis gets one accumulating matmul whose stationary [128, 32] is zero
    except column c%32 == ones; the column sums land on PSUM partition
    c%32 of bank c//32 (fp32r matmuls must write dst partition block 0 -
    s3d3_mm_valid_dst_partition - so each 32-chunk block gets its own
    [32, 512] PSUM bank instead of a tile_position row offset).  The
    other 31 partitions of a block receive +0.0, so chained accumulation
    over all chunks/row-tiles is exact.
  * Moving/stationary dtype is float32r: full-rate 1 col/cycle on the PE
    (vs 4x derate for plain fp32), ~1e-4 relative rounding error - far
    inside the 2e-2 gate.  fp32r tiles are filled by plain HWDGE DMA
    from fp32r-typed DRAM (same bits as f32); the ones-stationaries are
    host-provided so nothing on-chip needs to round.
  * The DVE does almost nothing: one [128, 512] PSUM->SBUF copy at the
    end.  h/r batch sums ride a second PSUM bank via a ones-column f32
    matmul over SWDGE-loaded chunks (gpsimd ring, leaving both HWDGE
    rings to the t stream).
  * The last stripe is emitted as 4 quarter tiles so the final
    tile's matmul+copy+store tail is ~2 us instead of ~5 us.
"""

import numpy as np

B, N, D = 2048, 512, 128
NCORES = 8
FLAT = N * D                 # 65536 flattened (n, d) columns
B_LOC = B // NCORES          # 256 rows per core
NRT = B_LOC // 128           # 2 row-tiles of 128 rows
SW = 16384                   # stripe width (64 KB per partition)
NS = FLAT // SW              # 4 stripes

_BUILT = None
# test.py can inject {"trace": True, ...} here; harness path leaves it empty.
RUN_KWARGS = {}
LAST_RESULTS = None


def _build():
    from concourse import bacc, tile, mybir

    f32 = mybir.dt.float32
    f32r = mybir.dt.float32r
    nc = bacc.Bacc(
        "TRN2",
        target_bir_lowering=False,
        debug=False,
        enable_asserts=False,
        num_devices=NCORES,
    )
    t_in = nc.dram_tensor("t_shard", [B_LOC, FLAT], f32r, kind="ExternalInput").ap()
    w_in = nc.dram_tensor("w_ones", [128, 32 * 32], f32r, kind="ExternalInput").ap()
    h_in = nc.dram_tensor("h_shard", [B_LOC, D], f32, kind="ExternalInput").ap()
    r_in = nc.dram_tensor("r_shard", [B_LOC, D], f32, kind="ExternalInput").ap()
    out_t = nc.dram_tensor("out_t_part", [128, 512], f32, kind="ExternalOutput").ap()
    out_hr = nc.dram_tensor("out_hr_part", [2, D], f32, kind="ExternalOutput").ap()

    # (row_tile, col0, width); last stripe split 8192 + 4x2048 so the
    # final tile's matmul backlog + copy + store tail is ~2 us
    plan = [(r, s * SW, SW) for r in range(NRT) for s in range(NS)]
    last0 = (NS - 1) * SW
    plan = plan[:-1] + [(NRT - 1, last0, SW // 2)] + [
        (NRT - 1, last0 + SW // 2 + q * (SW // 8), SW // 8) for q in range(4)
    ]

    with tile.TileContext(nc) as tc:
        with (
            tc.tile_pool(name="wconst", bufs=1) as wpool,
            tc.tile_pool(name="loads", bufs=2) as loads,
            tc.tile_pool(name="tap", bufs=4) as tap,
            tc.tile_pool(name="hr", bufs=6) as hrpool,
            tc.tile_pool(name="res", bufs=1) as res,
            tc.tile_pool(name="acc", bufs=1, space="PSUM") as ppool,
            tc.tile_pool(name="acchr", bufs=1, space="PSUM") as ppool2,
        ):
            W = wpool.tile([128, 32, 32], f32r)   # ones-column stationaries
            Whr = wpool.tile([128, 256], f32)
            psums = [
                ppool.tile([32, 512], f32, name=f"psum_blk{b}") for b in range(4)
            ]
            psum_hr = ppool2.tile([128, D], f32)

            def emit_setup_and_hr():
                # Emitted after the first two t loads so their DMA issue
                # is never delayed.  W rides the sync HWDGE ring (shares
                # packet-interleaved bandwidth with the t stream); h/r
                # ride the SWDGE (gpsimd) ring.
                nc.sync.dma_start(
                    W[:], w_in[:, :].rearrange("p (v c) -> p v c", v=32)
                )
                # Whr is zero except column 128 == 1.0; Whr[:, 128-j:256-j]
                # is a [128, 128] stationary whose column j is all-ones.
                nc.vector.memset(Whr[:], 0.0)
                nc.vector.memset(Whr[:, 128:129], 1.0)
                chunks = []
                for row, src in ((0, h_in), (1, r_in)):
                    for c0 in range(0, B_LOC, 128):
                        ht = hrpool.tile([128, D], f32)
                        nc.gpsimd.dma_start(ht[:], src[c0 : c0 + 128, :])
                        chunks.append((row, ht))
                for i, (row, ht) in enumerate(chunks):
                    nc.tensor.matmul(
                        psum_hr[:],
                        Whr[:, 128 - row : 256 - row],
                        ht[:],
                        start=(i == 0),
                        stop=(i == len(chunks) - 1),
                    )

            def emit_dma(k, r, c0, width):
                # full/half stripes share the 64KB "loads" slots (same
                # name -> same tag); the 2048-wide taper tiles get small
                # dedicated slots
                pool = loads if width >= SW // 2 else tap
                tl = pool.tile([128, width], f32r, name="tl" if width >= SW // 2 else "tap_tl")
                dma = nc.sync if k % 2 == 0 else nc.scalar
                dma.dma_start(tl[:], t_in[128 * r : 128 * r + 128, c0 : c0 + width])
                return tl

            def emit_matmuls(tl, r, c0, width):
                for i in range(width // 512):
                    c = c0 // 512 + i          # global chunk id 0..127
                    j, blk = c % 32, c // 32
                    nc.tensor.matmul(
                        psums[blk][:, :],
                        W[:, j, :],
                        tl[:, 512 * i : 512 * (i + 1)],
                        start=(r == 0 and j == 0),
                        stop=(r == NRT - 1 and j == 31),
                        tile_position=(0, 0),
                    )

            # The first two tile DMAs go out before anything else so the
            # t stream starts immediately; their matmuls are emitted AFTER
            # emit_setup_and_hr() because the Tile dependency tracker works
            # in emission order - a matmul emitted before W's DMA would
            # race the weight load (first-run garbage in block 0).
            head = [(k, *plan[k]) for k in range(2)]
            head_tiles = [emit_dma(k, r, c0, w_) for (k, r, c0, w_) in head]
            emit_setup_and_hr()
            for tl, (k, r, c0, w_) in zip(head_tiles, head):
                emit_matmuls(tl, r, c0, w_)
            for k, (r, c0, width) in enumerate(plan):
                if k < 2:
                    continue
                tl = emit_dma(k, r, c0, width)
                emit_matmuls(tl, r, c0, width)

            res_t = res.tile([128, 512], f32)
            for b in range(4):
                nc.vector.tensor_copy(res_t[32 * b : 32 * b + 32, :], psums[b][:, :])
            nc.sync.dma_start(out_t[:], res_t[:])

            res_hr = res.tile([2, D], f32)
            nc.vector.tensor_copy(res_hr[:], psum_hr[0:2, :])
            nc.scalar.dma_start(out_hr[:], res_hr[:])

    nc.compile()
    return nc


def _get_built():
    global _BUILT
    if _BUILT is None:
        _BUILT = _build()
    return _BUILT


def _w_ones():
    w = np.zeros((128, 32, 32), dtype=np.float32)
    for j in range(32):
        w[:, j, j] = 1.0
    return np.ascontiguousarray(w.reshape(128, 1024))


def kernel(h, r, t, w_i, w_j, w_k):
    global LAST_RESULTS
    from concourse import bass_utils

    nc = _get_built()
    t2 = np.ascontiguousarray(t, dtype=np.float32).reshape(B, FLAT)
    h = np.ascontiguousarray(h, dtype=np.float32)
    r = np.ascontiguousarray(r, dtype=np.float32)
    wv = _w_ones()

    in_maps = []
    for c in range(NCORES):
        s, e = c * B_LOC, (c + 1) * B_LOC
        in_maps.append(
            {
                "t_shard": t2[s:e],
                "h_shard": h[s:e],
                "r_shard": r[s:e],
                "w_ones": wv,
            }
        )
    results = bass_utils.run_bass_kernel_spmd(
        nc, in_maps, core_ids=list(range(NCORES)), **RUN_KWARGS
    )
    LAST_RESULTS = results

    sum_t = np.zeros(FLAT, dtype=np.float64)
    sum_h = np.zeros(D, dtype=np.float64)
    sum_r = np.zeros(D, dtype=np.float64)
    for c in range(NCORES):
        sum_t += results.results[c]["out_t_part"].reshape(FLAT)
        sum_h += results.results[c]["out_hr_part"][0]
        sum_r += results.results[c]["out_hr_part"][1]

    out = np.empty((N, 3 * D), dtype=np.float32)
    out[:, 0:D] = sum_h.astype(np.float32)[None, :]
    out[:, D : 2 * D] = sum_r.astype(np.float32)[None, :]
    out[:, 2 * D :] = sum_t.astype(np.float32).reshape(N, D)
    return out


# revision 14
# speedup vs baseline: 1.0448x; 1.0448x over previous
"""Trainium2 Bass kernel for nn_BatchGraphEncoder (gnn_message_passing).

Math note: the reference's segment softmax uses B unique segment ids
(groups of size 1), so alpha == 1.0 bit-exactly.  The output reduces to
pure batch sums:

    out[:,   0:128] = sum_b h[b,:]      (broadcast over the N=512 rows)
    out[:, 128:256] = sum_b r[b,:]      (broadcast)
    out[:, 256:384] = sum_b t[b,:,:]    ([512, 128])

Memory-bound reduction over B=2048 dominated by reading t (512 MB).
B is sharded equally across the 8 cores (256 rows each); the 8 tiny
partials are summed on the host.

Per-core strategy (v2, TensorEngine reduction):
  * Rows go on SBUF partitions: stripe tiles [128 rows, 8192 cols] give
    32 KB contiguous DMA runs per partition (vs 2 KB for the
    columns-on-partitions layout), lifting the 16 SDMA engines from
    ~80 ns/2KB packet to ~400 GB/s.  (64 KB runs with 2 slots were
    tried and regress: slot recycling waits on a whole tile's matmuls
    and the 2-deep pipeline starves the DMA every other tile.)
  * The batch sum contracts the partition dim, which is exactly what the
    PE does.  Each 512-column chunk c of the flattened [512*128] feature
    axis gets one accumulating matmul whose stationary [128, 32] is zero
    except column c%32 == ones; the column sums land on PSUM partition
    c%32 of bank c//32 (fp32r matmuls must write dst partition block 0 -
    s3d3_mm_valid_dst_partition - so each 32-chunk block gets its own
    [32, 512] PSUM bank instead of a tile_position row offset).  The
    other 31 partitions of a block receive +0.0, so chained accumulation
    over all chunks/row-tiles is exact.
  * Moving/stationary dtype is float32r: full-rate 1 col/cycle on the PE
    (vs 4x derate for plain fp32), ~1e-4 relative rounding error - far
    inside the 2e-2 gate.  fp32r tiles are filled by plain HWDGE DMA
    from fp32r-typed DRAM (same bits as f32); the ones-stationaries are
    host-provided so nothing on-chip needs to round.
  * The DVE does almost nothing: one [128, 512] PSUM->SBUF copy at the
    end.  h/r batch sums ride a second PSUM bank via a ones-column f32
    matmul over SWDGE-loaded chunks (gpsimd ring, leaving both HWDGE
    rings to the t stream).
  * The last stripe is emitted as 4 quarter tiles so the final
    tile's matmul+copy+store tail is ~2 us instead of ~5 us.
"""

import numpy as np

B, N, D = 2048, 512, 128
NCORES = 8
FLAT = N * D                 # 65536 flattened (n, d) columns
B_LOC = B // NCORES          # 256 rows per core
NRT = B_LOC // 128           # 2 row-tiles of 128 rows
SW = 8192                    # stripe width (32 KB per partition)
NS = FLAT // SW              # 8 stripes

_BUILT = None
# test.py can inject {"trace": True, ...} here; harness path leaves it empty.
RUN_KWARGS = {}
LAST_RESULTS = None


def _build():
    from concourse import bacc, tile, mybir

    f32 = mybir.dt.float32
    f32r = mybir.dt.float32r
    nc = bacc.Bacc(
        "TRN2",
        target_bir_lowering=False,
        debug=False,
        enable_asserts=False,
        num_devices=NCORES,
    )
    t_in = nc.dram_tensor("t_shard", [B_LOC, FLAT], f32r, kind="ExternalInput").ap()
    w_in = nc.dram_tensor("w_ones", [128, 32 * 32], f32r, kind="ExternalInput").ap()
    h_in = nc.dram_tensor("h_shard", [B_LOC, D], f32, kind="ExternalInput").ap()
    r_in = nc.dram_tensor("r_shard", [B_LOC, D], f32, kind="ExternalInput").ap()
    out_t = nc.dram_tensor("out_t_part", [128, 512], f32, kind="ExternalOutput").ap()
    out_hr = nc.dram_tensor("out_hr_part", [2, D], f32, kind="ExternalOutput").ap()

    # (row_tile, col0, width); last stripe split 8192 + 4x2048 so the
    # final tile's matmul backlog + copy + store tail is ~2 us
    plan = [(r, s * SW, SW) for r in range(NRT) for s in range(NS)]
    last0 = (NS - 1) * SW
    plan = plan[:-1] + [(NRT - 1, last0, SW // 2)] + [
        (NRT - 1, last0 + SW // 2 + q * (SW // 8), SW // 8) for q in range(4)
    ]

    with tile.TileContext(nc) as tc:
        with (
            tc.tile_pool(name="wconst", bufs=1) as wpool,
            tc.tile_pool(name="loads", bufs=4) as loads,
            tc.tile_pool(name="tap", bufs=4) as tap,
            tc.tile_pool(name="hr", bufs=6) as hrpool,
            tc.tile_pool(name="res", bufs=1) as res,
            tc.tile_pool(name="acc", bufs=1, space="PSUM") as ppool,
            tc.tile_pool(name="acchr", bufs=1, space="PSUM") as ppool2,
        ):
            W = wpool.tile([128, 32, 32], f32r)   # ones-column stationaries
            Whr = wpool.tile([128, 256], f32)
            psums = [
                ppool.tile([32, 512], f32, name=f"psum_blk{b}") for b in range(4)
            ]
            psum_hr = ppool2.tile([128, D], f32)

            def emit_setup_and_hr():
                # Emitted after the first two t loads so their DMA issue
                # is never delayed.  W rides the sync HWDGE ring (shares
                # packet-interleaved bandwidth with the t stream); h/r
                # ride the SWDGE (gpsimd) ring.
                nc.sync.dma_start(
                    W[:], w_in[:, :].rearrange("p (v c) -> p v c", v=32)
                )
                # Whr is zero except column 128 == 1.0; Whr[:, 128-j:256-j]
                # is a [128, 128] stationary whose column j is all-ones.
                nc.vector.memset(Whr[:], 0.0)
                nc.vector.memset(Whr[:, 128:129], 1.0)
                chunks = []
                for row, src in ((0, h_in), (1, r_in)):
                    for c0 in range(0, B_LOC, 128):
                        ht = hrpool.tile([128, D], f32)
                        nc.gpsimd.dma_start(ht[:], src[c0 : c0 + 128, :])
                        chunks.append((row, ht))
                for i, (row, ht) in enumerate(chunks):
                    nc.tensor.matmul(
                        psum_hr[:],
                        Whr[:, 128 - row : 256 - row],
                        ht[:],
                        start=(i == 0),
                        stop=(i == len(chunks) - 1),
                    )

            def emit_dma(k, r, c0, width):
                # full/half stripes share the 64KB "loads" slots (same
                # name -> same tag); the 2048-wide taper tiles get small
                # dedicated slots
                pool = loads if width >= SW // 2 else tap
                tl = pool.tile([128, width], f32r, name="tl" if width >= SW // 2 else "tap_tl")
                dma = nc.sync if k % 2 == 0 else nc.scalar
                dma.dma_start(tl[:], t_in[128 * r : 128 * r + 128, c0 : c0 + width])
                return tl

            def emit_matmuls(tl, r, c0, width):
                for i in range(width // 512):
                    c = c0 // 512 + i          # global chunk id 0..127
                    j, blk = c % 32, c // 32
                    nc.tensor.matmul(
                        psums[blk][:, :],
                        W[:, j, :],
                        tl[:, 512 * i : 512 * (i + 1)],
                        start=(r == 0 and j == 0),
                        stop=(r == NRT - 1 and j == 31),
                        tile_position=(0, 0),
                    )

            # The first two tile DMAs go out before anything else so the
            # t stream starts immediately; their matmuls are emitted AFTER
            # emit_setup_and_hr() because the Tile dependency tracker works
            # in emission order - a matmul emitted before W's DMA would
            # race the weight load (first-run garbage in block 0).
            head = [(k, *plan[k]) for k in range(2)]
            head_tiles = [emit_dma(k, r, c0, w_) for (k, r, c0, w_) in head]
            emit_setup_and_hr()
            for tl, (k, r, c0, w_) in zip(head_tiles, head):
                emit_matmuls(tl, r, c0, w_)
            for k, (r, c0, width) in enumerate(plan):
                if k < 2:
                    continue
                tl = emit_dma(k, r, c0, width)
                emit_matmuls(tl, r, c0, width)

            res_t = res.tile([128, 512], f32)
            for b in range(4):
                nc.vector.tensor_copy(res_t[32 * b : 32 * b + 32, :], psums[b][:, :])
            nc.sync.dma_start(out_t[:], res_t[:])

            res_hr = res.tile([2, D], f32)
            nc.vector.tensor_copy(res_hr[:], psum_hr[0:2, :])
            nc.scalar.dma_start(out_hr[:], res_hr[:])

    nc.compile()
    return nc


def _get_built():
    global _BUILT
    if _BUILT is None:
        _BUILT = _build()
    return _BUILT


def _w_ones():
    w = np.zeros((128, 32, 32), dtype=np.float32)
    for j in range(32):
        w[:, j, j] = 1.0
    return np.ascontiguousarray(w.reshape(128, 1024))


def kernel(h, r, t, w_i, w_j, w_k):
    global LAST_RESULTS
    from concourse import bass_utils

    nc = _get_built()
    t2 = np.ascontiguousarray(t, dtype=np.float32).reshape(B, FLAT)
    h = np.ascontiguousarray(h, dtype=np.float32)
    r = np.ascontiguousarray(r, dtype=np.float32)
    wv = _w_ones()

    in_maps = []
    for c in range(NCORES):
        s, e = c * B_LOC, (c + 1) * B_LOC
        in_maps.append(
            {
                "t_shard": t2[s:e],
                "h_shard": h[s:e],
                "r_shard": r[s:e],
                "w_ones": wv,
            }
        )
    results = bass_utils.run_bass_kernel_spmd(
        nc, in_maps, core_ids=list(range(NCORES)), **RUN_KWARGS
    )
    LAST_RESULTS = results

    sum_t = np.zeros(FLAT, dtype=np.float64)
    sum_h = np.zeros(D, dtype=np.float64)
    sum_r = np.zeros(D, dtype=np.float64)
    for c in range(NCORES):
        sum_t += results.results[c]["out_t_part"].reshape(FLAT)
        sum_h += results.results[c]["out_hr_part"][0]
        sum_r += results.results[c]["out_hr_part"][1]

    out = np.empty((N, 3 * D), dtype=np.float32)
    out[:, 0:D] = sum_h.astype(np.float32)[None, :]
    out[:, D : 2 * D] = sum_r.astype(np.float32)[None, :]
    out[:, 2 * D :] = sum_t.astype(np.float32).reshape(N, D)
    return out


# revision 17
# speedup vs baseline: 1.0827x; 1.0363x over previous
"""Trainium2 Bass kernel for nn_BatchGraphEncoder (gnn_message_passing).

Math note: the reference's segment softmax uses B unique segment ids
(groups of size 1), so alpha == 1.0 bit-exactly.  The output reduces to
pure batch sums:

    out[:,   0:128] = sum_b h[b,:]      (broadcast over the N=512 rows)
    out[:, 128:256] = sum_b r[b,:]      (broadcast)
    out[:, 256:384] = sum_b t[b,:,:]    ([512, 128])

Memory-bound reduction over B=2048 dominated by reading t (512 MB).
B is sharded equally across the 8 cores (256 rows each); the 8 tiny
partials are summed on the host.

Per-core strategy (v2, TensorEngine reduction):
  * Rows go on SBUF partitions: stripe tiles [128 rows, 8192 cols] give
    32 KB contiguous DMA runs per partition (vs 2 KB for the
    columns-on-partitions layout), lifting the 16 SDMA engines from
    ~80 ns/2KB packet to ~400 GB/s.  (64 KB runs with 2 slots were
    tried and regress: slot recycling waits on a whole tile's matmuls
    and the 2-deep pipeline starves the DMA every other tile.)
  * The batch sum contracts the partition dim, which is exactly what the
    PE does.  Each 512-column chunk c of the flattened [512*128] feature
    axis gets one accumulating matmul whose stationary [128, 32] is zero
    except column c%32 == ones; the column sums land on PSUM partition
    c%32 of bank c//32 (fp32r matmuls must write dst partition block 0 -
    s3d3_mm_valid_dst_partition - so each 32-chunk block gets its own
    [32, 512] PSUM bank instead of a tile_position row offset).  The
    other 31 partitions of a block receive +0.0, so chained accumulation
    over all chunks/row-tiles is exact.
  * Moving/stationary dtype is float32r: full-rate 1 col/cycle on the PE
    (vs 4x derate for plain fp32), ~1e-4 relative rounding error - far
    inside the 2e-2 gate.  fp32r tiles are filled by plain HWDGE DMA
    from fp32r-typed DRAM (same bits as f32); the ones-stationaries are
    host-provided so nothing on-chip needs to round.
  * The DVE does almost nothing: one [128, 512] PSUM->SBUF copy at the
    end.  h/r batch sums ride a second PSUM bank via a ones-column f32
    matmul over SWDGE-loaded chunks (gpsimd ring, leaving both HWDGE
    rings to the t stream).
  * The last stripe is emitted as 4 quarter tiles so the final
    tile's matmul+copy+store tail is ~2 us instead of ~5 us.
"""

import numpy as np

B, N, D = 2048, 512, 128
NCORES = 8
FLAT = N * D                 # 65536 flattened (n, d) columns
B_LOC = B // NCORES          # 256 rows per core
NRT = B_LOC // 128           # 2 row-tiles of 128 rows
SW = 8192                    # stripe width (32 KB per partition)
NS = FLAT // SW              # 8 stripes

_BUILT = None
# test.py can inject {"trace": True, ...} here; harness path leaves it empty.
RUN_KWARGS = {}
LAST_RESULTS = None


def _build():
    from concourse import bacc, tile, mybir

    f32 = mybir.dt.float32
    f32r = mybir.dt.float32r
    nc = bacc.Bacc(
        "TRN2",
        target_bir_lowering=False,
        debug=False,
        enable_asserts=False,
        num_devices=NCORES,
    )
    t_in = nc.dram_tensor("t_shard", [B_LOC, FLAT], f32r, kind="ExternalInput").ap()
    w_in = nc.dram_tensor("w_ones", [128, 32 * 32], f32r, kind="ExternalInput").ap()
    h_in = nc.dram_tensor("h_shard", [B_LOC, D], f32, kind="ExternalInput").ap()
    r_in = nc.dram_tensor("r_shard", [B_LOC, D], f32, kind="ExternalInput").ap()
    out_t = nc.dram_tensor("out_t_part", [128, 512], f32, kind="ExternalOutput").ap()
    out_hr = nc.dram_tensor("out_hr_part", [2, D], f32, kind="ExternalOutput").ap()

    # (row_tile, col0, width); last stripe split 8192 + 4x2048 so the
    # final tile's matmul backlog + copy + store tail is ~2 us
    plan = [(r, s * SW, SW) for r in range(NRT) for s in range(NS)]
    last0 = (NS - 1) * SW
    plan = plan[:-1] + [(NRT - 1, last0, SW // 2)] + [
        (NRT - 1, last0 + SW // 2 + q * (SW // 8), SW // 8) for q in range(4)
    ]

    with tile.TileContext(nc) as tc:
        with (
            tc.tile_pool(name="wconst", bufs=1) as wpool,
            tc.tile_pool(name="loads", bufs=4) as loads,
            tc.tile_pool(name="tap", bufs=4) as tap,
            tc.tile_pool(name="hr", bufs=6) as hrpool,
            tc.tile_pool(name="res", bufs=1) as res,
            tc.tile_pool(name="acc", bufs=1, space="PSUM") as ppool,
            tc.tile_pool(name="acchr", bufs=1, space="PSUM") as ppool2,
        ):
            W = wpool.tile([128, 32, 32], f32r)   # ones-column stationaries
            Whr = wpool.tile([128, 256], f32)
            psums = [
                ppool.tile([32, 512], f32, name=f"psum_blk{b}") for b in range(4)
            ]
            psum_hr = ppool2.tile([128, D], f32)

            def emit_setup_and_hr():
                # Emitted after the first two t loads so their DMA issue
                # is never delayed.  W rides the sync HWDGE ring (shares
                # packet-interleaved bandwidth with the t stream); h/r
                # ride the SWDGE (gpsimd) ring.
                nc.scalar.dma_start(
                    W[:], w_in[:, :].rearrange("p (v c) -> p v c", v=32)
                )
                # Whr is zero except column 128 == 1.0; Whr[:, 128-j:256-j]
                # is a [128, 128] stationary whose column j is all-ones.
                nc.vector.memset(Whr[:], 0.0)
                nc.vector.memset(Whr[:, 128:129], 1.0)
                chunks = []
                for row, src in ((0, h_in), (1, r_in)):
                    for c0 in range(0, B_LOC, 128):
                        ht = hrpool.tile([128, D], f32)
                        nc.gpsimd.dma_start(ht[:], src[c0 : c0 + 128, :])
                        chunks.append((row, ht))
                for i, (row, ht) in enumerate(chunks):
                    nc.tensor.matmul(
                        psum_hr[:],
                        Whr[:, 128 - row : 256 - row],
                        ht[:],
                        start=(i == 0),
                        stop=(i == len(chunks) - 1),
                    )

            def emit_dma(k, r, c0, width):
                # full/half stripes share the 64KB "loads" slots (same
                # name -> same tag); the 2048-wide taper tiles get small
                # dedicated slots
                pool = loads if width >= SW // 2 else tap
                tl = pool.tile([128, width], f32r, name="tl" if width >= SW // 2 else "tap_tl")
                # All t tiles on the sync ring: one FIFO queue row keeps the
                # SDMA engines from round-robin switching between the two
                # HWDGE rows at packet granularity; the DGE easily keeps
                # ahead at 128 descriptors per tile.
                nc.sync.dma_start(tl[:], t_in[128 * r : 128 * r + 128, c0 : c0 + width])
                return tl

            def emit_matmuls(tl, r, c0, width):
                for i in range(width // 512):
                    c = c0 // 512 + i          # global chunk id 0..127
                    j, blk = c % 32, c // 32
                    nc.tensor.matmul(
                        psums[blk][:, :],
                        W[:, j, :],
                        tl[:, 512 * i : 512 * (i + 1)],
                        start=(r == 0 and j == 0),
                        stop=(r == NRT - 1 and j == 31),
                        tile_position=(0, 0),
                    )

            # The first two tile DMAs go out before anything else so the
            # t stream starts immediately; their matmuls are emitted AFTER
            # emit_setup_and_hr() because the Tile dependency tracker works
            # in emission order - a matmul emitted before W's DMA would
            # race the weight load (first-run garbage in block 0).
            head = [(k, *plan[k]) for k in range(2)]
            head_tiles = [emit_dma(k, r, c0, w_) for (k, r, c0, w_) in head]
            emit_setup_and_hr()
            for tl, (k, r, c0, w_) in zip(head_tiles, head):
                emit_matmuls(tl, r, c0, w_)
            for k, (r, c0, width) in enumerate(plan):
                if k < 2:
                    continue
                tl = emit_dma(k, r, c0, width)
                emit_matmuls(tl, r, c0, width)

            res_t = res.tile([128, 512], f32)
            for b in range(4):
                nc.vector.tensor_copy(res_t[32 * b : 32 * b + 32, :], psums[b][:, :])
            nc.scalar.dma_start(out_t[:], res_t[:])

            res_hr = res.tile([2, D], f32)
            nc.vector.tensor_copy(res_hr[:], psum_hr[0:2, :])
            nc.scalar.dma_start(out_hr[:], res_hr[:])

    nc.compile()
    return nc


def _get_built():
    global _BUILT
    if _BUILT is None:
        _BUILT = _build()
    return _BUILT


def _w_ones():
    w = np.zeros((128, 32, 32), dtype=np.float32)
    for j in range(32):
        w[:, j, j] = 1.0
    return np.ascontiguousarray(w.reshape(128, 1024))


def kernel(h, r, t, w_i, w_j, w_k):
    global LAST_RESULTS
    from concourse import bass_utils

    nc = _get_built()
    t2 = np.ascontiguousarray(t, dtype=np.float32).reshape(B, FLAT)
    h = np.ascontiguousarray(h, dtype=np.float32)
    r = np.ascontiguousarray(r, dtype=np.float32)
    wv = _w_ones()

    in_maps = []
    for c in range(NCORES):
        s, e = c * B_LOC, (c + 1) * B_LOC
        in_maps.append(
            {
                "t_shard": t2[s:e],
                "h_shard": h[s:e],
                "r_shard": r[s:e],
                "w_ones": wv,
            }
        )
    results = bass_utils.run_bass_kernel_spmd(
        nc, in_maps, core_ids=list(range(NCORES)), **RUN_KWARGS
    )
    LAST_RESULTS = results

    sum_t = np.zeros(FLAT, dtype=np.float64)
    sum_h = np.zeros(D, dtype=np.float64)
    sum_r = np.zeros(D, dtype=np.float64)
    for c in range(NCORES):
        sum_t += results.results[c]["out_t_part"].reshape(FLAT)
        sum_h += results.results[c]["out_hr_part"][0]
        sum_r += results.results[c]["out_hr_part"][1]

    out = np.empty((N, 3 * D), dtype=np.float32)
    out[:, 0:D] = sum_h.astype(np.float32)[None, :]
    out[:, D : 2 * D] = sum_r.astype(np.float32)[None, :]
    out[:, 2 * D :] = sum_t.astype(np.float32).reshape(N, D)
    return out
